# revision 33
# baseline (speedup 1.0000x reference)
"""HGT layer kernel for 8 Trainium2 NeuronCores.

Strategy (dst-sharded graph parallel, transfer-optimized):
  - Host folds relation transforms / priors / skip gate into effective
    weights, and folds the skip term into a pre-scaled h upload:
    h_pre = (1-alpha)*h + alpha*ba, with projection weights rescaled by
    1/(1-alpha) so all device math runs off the single h_pre upload.
  - ONE packed fp16 input param per core: [h_pre | 1/8 weight slice |
    wrapped src indices | wrapped one-hot column ids | vn gather ids].
    Weights are AllGathered on device (uploaded once, not 8x). The one-hot
    aggregation matrix is generated on device by gathering rows of an
    identity table, q-gather indices are derived on device from the column
    ids, h^T is derived via DMA-transpose, and the skip add is done with an
    identity matmul.
  - Each core owns N/8=2500 destination nodes and their incoming edges.
  - Device: project q/kv for own nodes (fp16), AllGather kv table, then for
    each window of <=128 dst nodes (2048 edge slots): dma_gather kv[src] and
    q[dst] rows, DVE dot-product scores, ACT exp, PE onehot-matmul
    aggregation of [messages | exp] into PSUM, normalize, flush.
  - Final: transpose-gather normalized agg -> output projection + skip,
    fp16 output (halves result + donated-zero traffic).
"""

import math
import numpy as np

import jax

# Persistent XLA compilation cache: the axon PJRT plugin supports executable
# serialization, so warm calls skip the per-call neuronx recompile (~0.2s).
try:
    jax.config.update("jax_compilation_cache_dir", "/tmp/jaxcache")
    jax.config.update("jax_persistent_cache_min_compile_time_secs", 0.0)
    jax.config.update("jax_persistent_cache_min_entry_size_bytes", 0)
except Exception:
    pass

import concourse.bacc as bacc
import concourse.tile as tile
import concourse.bass as bass
from concourse import mybir
from concourse.masks import make_identity
from concourse.bass_utils import run_bass_kernel_spmd

N = 20000
E = 320000
D = 256
H = 8
DK = 32
NCORES = 8
NPC = N // NCORES          # 2500 nodes per core
NTN = 2560                 # padded nodes per core (20 tiles of 128)
NTILES = NTN // 128        # 20
WSLOTS = 2048              # edge slots per window
WCH = WSLOTS // 128        # 16 chunks per window
WSPAN = 128                # max dst nodes per window
WROWS = 1032               # weight block rows (8 x 129)
WSL = WROWS // NCORES      # 129 rows per core

F16 = mybir.dt.float16
F32 = mybir.dt.float32
I16 = mybir.dt.int16
I8 = mybir.dt.int8
DO = D + 2                 # int8 output row: 256 data + 2 scale bytes
MAGIC = 12582912.0         # 1.5 * 2**23: f32 round-to-nearest-integer trick

_cache = {}
LAST_RESULTS = None
LAST_EXEC_NS = None


def _layout(NW):
    """Row offsets inside the packed [R, 256] fp16 param."""
    NWp = 4 * ((NW + 3) // 4)          # pad window count so wrapped idx
    HS0 = NTN // 2                     # blocks are whole 256-col rows
    W0 = HS0 + 16                      # h int8 block, then [128, 32] scales
    S0 = W0 + WSL                      # sidx (16-wrapped, [16, NWp*128] i16)
    C0 = S0 + 8 * NWp                  # col ids (16-wrapped, int8)
    V0 = C0 + 4 * NWp                  # vidx + wstarts ([16, 256] i16)
    R = V0 + 16
    return NWp, HS0, W0, S0, C0, V0, R


def _build(NW, use_bias, use_fbias):
    NWp, HS0, W0, S0, C0, V0, R = _layout(NW)
    Lw = NWp * 128
    nc = bacc.Bacc()
    p1 = nc.declare_dram_parameter("p1", [R, D], F16, isOutput=False)
    outp = nc.declare_dram_parameter("out", [NPC, DO], I8, isOutput=True)

    with tile.TileContext(nc) as tc:
        with (
            tc.tile_pool(name="const", bufs=1) as constp,
            tc.tile_pool(name="dram", bufs=1, space="DRAM") as dram,
            tc.tile_pool(name="proj", bufs=3) as projp,
            tc.tile_pool(name="psum", bufs=2, space="PSUM") as psump,
            tc.tile_pool(name="edge", bufs=2) as edgep,
            tc.tile_pool(name="fin", bufs=2) as finp,
        ):
            q_tab = dram.tile([NTN, D], F16)
            kv_slice = dram.tile([NTN, 2 * D], F16)
            kv_full = nc.dram_tensor(
                "kv_full", [NCORES * NTN, 2 * D], F16, addr_space="Shared")
            w_slice = dram.tile([WSL, D], F16)
            w_full = nc.dram_tensor("w_full", [WROWS, D], F16, addr_space="Shared")
            ioh = dram.tile([256, 128], F16)   # [identity | zeros] for onehot
            vn = dram.tile([NW * 128, D], F16)

            # ---- weight slice -> AllGather full weight block ----
            nc.sync.dma_start(w_slice[:], p1[W0:W0 + WSL, :])
            nc.gpsimd.collective_compute(
                "AllGather",
                mybir.AluOpType.bypass,
                replica_groups=[list(range(NCORES))],
                ins=[w_slice.opt()],
                outs=[w_full[:]],
            )

            # ---- constants ----
            # h int8 rows + per-node f16 scales -> dequantized h_pre tiles
            hq_sb = constp.tile([128, NTILES, D], I8)
            nc.sync.dma_start(
                hq_sb[:].rearrange("p t c -> p (t c)"),
                p1[0:NTN // 2, :].rearrange("(p r) c -> p (r c)", p=128)
                .bitcast(I8))
            hs_sb = constp.tile([128, 32], F16)
            nc.sync.dma_start(
                hs_sb[:],
                p1[HS0:HS0 + 16, :].rearrange("q (g c) -> (q g) c", g=8))
            hs32 = constp.tile([128, 32], F32)
            nc.vector.tensor_copy(hs32[:], hs_sb[:])
            hp_sb = constp.tile([128, NTILES, D], F16)
            for t in range(NTILES):
                nc.scalar.mul(hp_sb[:, t, :], hq_sb[:, t, :], hs32[:, t:t + 1])
            # h^T via SBUF->SBUF DMA-transpose of the dequantized tiles
            hT_sb = constp.tile([128, 2, NTN], F16)
            for t in range(NTILES):
                for j in (0, 1):
                    nc.sync.dma_start(
                        hT_sb[:, j, t * 128:(t + 1) * 128],
                        hp_sb[:, t, j * 128:(j + 1) * 128],
                        transpose=True,
                    )
            wq_sb = constp.tile([128, 2, D], F16)
            for j in (0, 1):
                nc.sync.dma_start(
                    wq_sb[:, j, :], w_full[j * 128:(j + 1) * 128, :])
            wkv_sb = constp.tile([128, 2, 2 * D], F16)
            for j in (0, 1):
                for hh in (0, 1):
                    r0 = 256 + (j * 2 + hh) * 128
                    nc.sync.dma_start(
                        wkv_sb[:, j, hh * D:(hh + 1) * D], w_full[r0:r0 + 128, :])
            wa_sb = constp.tile([128, 2, D], F16)
            for j in (0, 1):
                r0 = 768 + j * 128
                nc.sync.dma_start(wa_sb[:, j, :], w_full[r0:r0 + 128, :])
            # wrapped gather indices: upload [16, L], replicate to 128 parts
            sidx_sb = constp.tile([128, Lw], I16)
            sidx_v = p1[S0:S0 + 8 * NWp, :].rearrange(
                "(p r) c -> p (r c)", p=16).bitcast(I16)
            colw8_sb = constp.tile([128, Lw], I8)
            colw_v = p1[C0:C0 + 4 * NWp, :].rearrange(
                "(p r) c -> p (r c)", p=16).bitcast(I8)
            vidx_sb = constp.tile([128, 256], I16)
            vidx_v = p1[V0:V0 + 16, :].rearrange(
                "(p r) c -> p (r c)", p=16).bitcast(I16)
            for g in range(8):
                nc.sync.dma_start(sidx_sb[16 * g:16 * (g + 1), :], sidx_v)
                nc.sync.dma_start(colw8_sb[16 * g:16 * (g + 1), :], colw_v)
                nc.sync.dma_start(vidx_sb[16 * g:16 * (g + 1), :], vidx_v)
            # int8 -128 pad marker sign-extends; mask to 128 (the zero row
            # of the one-hot table)
            colwx_sb = constp.tile([128, Lw], I16)
            nc.vector.tensor_copy(colwx_sb[:], colw8_sb[:])
            colw_sb = constp.tile([128, Lw], I16)
            nc.vector.tensor_scalar(
                colw_sb[:], colwx_sb[:], 255, None,
                op0=mybir.AluOpType.bitwise_and)
            # q gather ids: window-start (stashed in vidx block) + col id
            qidx_sb = constp.tile([128, Lw], I16)
            for w in range(NW):
                csl = slice(w * 128, (w + 1) * 128)
                nc.vector.tensor_add(
                    qidx_sb[:, csl], colw_sb[:, csl],
                    vidx_sb[:, 160 + w:161 + w].broadcast_to([128, 128]))
            nc.vector.tensor_scalar_min(
                qidx_sb[:, 0:NW * 128], qidx_sb[:, 0:NW * 128], NTN - 1)
            # identity-table for onehot generation (row 128 = all zeros)
            ident = constp.tile([128, 128], F16)
            make_identity(nc, ident[:])
            zeros_sb = constp.tile([128, 128], F16)
            nc.vector.memset(zeros_sb[:], 0.0)
            nc.sync.dma_start(ioh[0:128, :], ident[:])
            nc.sync.dma_start(ioh[128:256, :], zeros_sb[:])
            magic_sb = constp.tile([128, 1], F32)
            nc.vector.memset(magic_sb[:], MAGIC)
            if use_bias or use_fbias:
                ones_sb = constp.tile([1, 128], F16)
                nc.vector.memset(ones_sb[:], 1.0)
            if use_bias:
                bq_sb = constp.tile([1, D], F16)
                nc.sync.dma_start(bq_sb[:], w_full[1024:1025, :])
                bkv_sb = constp.tile([1, 2 * D], F16)
                nc.sync.dma_start(bkv_sb[:], w_full[1025:1027, :])
            if use_fbias:
                fb_sb = constp.tile([1, D], F16)
                nc.sync.dma_start(fb_sb[:], w_full[1027:1028, :])

            # ---- projection phase ----
            for nt in range(NTILES):
                sl = slice(nt * 128, (nt + 1) * 128)
                pkv = psump.tile([128, 2 * D], F32, tag="pkv")
                for j in (0, 1):
                    nc.tensor.matmul(
                        pkv[:], hT_sb[:, j, sl], wkv_sb[:, j, :],
                        start=(j == 0), stop=(j == 1 and not use_bias),
                    )
                if use_bias:
                    nc.tensor.matmul(pkv[:], ones_sb[:], bkv_sb[:], start=False, stop=True)
                kv_sb = projp.tile([128, 2 * D], F16, tag="kv")
                nc.vector.tensor_copy(kv_sb[:], pkv[:])
                nc.sync.dma_start(kv_slice[sl, :], kv_sb[:])

                pq = psump.tile([128, D], F32, tag="pq")
                for j in (0, 1):
                    nc.tensor.matmul(
                        pq[:], hT_sb[:, j, sl], wq_sb[:, j, :],
                        start=(j == 0), stop=(j == 1 and not use_bias),
                    )
                if use_bias:
                    nc.tensor.matmul(pq[:], ones_sb[:], bq_sb[:], start=False, stop=True)
                q_sb = projp.tile([128, D], F16, tag="q")
                nc.vector.tensor_copy(q_sb[:], pq[:])
                nc.sync.dma_start(q_tab[sl, :], q_sb[:])

            nc.gpsimd.collective_compute(
                "AllGather",
                mybir.AluOpType.bypass,
                replica_groups=[list(range(NCORES))],
                ins=[kv_slice.opt()],
                outs=[kv_full[:]],
            )

            # ---- edge phase ----
            for w in range(NW):
                csl = slice(w * 128, (w + 1) * 128)
                kvg = edgep.tile([128, WCH, 2 * D], F16, tag="kvg")
                nc.gpsimd.dma_gather(
                    kvg[:], kv_full[:], sidx_sb[:, csl],
                    num_idxs=WSLOTS, num_idxs_reg=WSLOTS, elem_size=2 * D,
                    single_packet=False,
                )
                qg = edgep.tile([128, WCH, D], F16, tag="qg")
                nc.gpsimd.dma_gather(
                    qg[:], q_tab[:], qidx_sb[:, csl],
                    num_idxs=WSLOTS, num_idxs_reg=WSLOTS, elem_size=D,
                    single_packet=False,
                )
                # one-hot [slot, dst-col]: gather identity-table rows
                oh_sb = edgep.tile([128, WCH, 128], F16, tag="oh")
                nc.gpsimd.dma_gather(
                    oh_sb[:], ioh[:], colw_sb[:, csl],
                    num_idxs=WSLOTS, num_idxs_reg=WSLOTS, elem_size=128,
                    single_packet=False,
                )

                prod = edgep.tile([128, WCH, D], F16, tag="prod")
                nc.vector.tensor_mul(prod[:], qg[:], kvg[:, :, 0:D])
                scores = edgep.tile([128, WCH, H], F32, tag="sc")
                nc.vector.tensor_reduce(
                    scores[:],
                    prod[:].rearrange("p c (h k) -> p c h k", h=H),
                    axis=mybir.AxisListType.X,
                    op=mybir.AluOpType.add,
                )
                msgz = edgep.tile([128, WCH, D + H], F16, tag="msgz")
                nc.scalar.activation(
                    msgz[:, :, D:D + H], scores[:], mybir.ActivationFunctionType.Exp
                )
                nc.vector.tensor_mul(
                    msgz[:, :, 0:D].rearrange("p c (h k) -> p c h k", h=H),
                    kvg[:, :, D:2 * D].rearrange("p c (h k) -> p c h k", h=H),
                    msgz[:, :, D:D + H].broadcast_to([128, WCH, H, DK]),
                )
                pw = psump.tile([128, D + H], F32, tag="pw")
                for i in range(WCH):
                    nc.tensor.matmul(
                        pw[:], oh_sb[:, i, :], msgz[:, i, :],
                        start=(i == 0), stop=(i == WCH - 1),
                    )
                zr = finp.tile([128, H], F32, tag="zr")
                nc.vector.tensor_scalar_add(zr[:], pw[:, D:D + H], 1e-30)
                zrec = finp.tile([128, H], F32, tag="zrec")
                nc.vector.reciprocal(zrec[:], zr[:])
                vb = finp.tile([128, D], F16, tag="vb")
                nc.vector.tensor_mul(
                    vb[:].rearrange("p (h k) -> p h k", h=H),
                    pw[:, 0:D].rearrange("p (h k) -> p h k", h=H),
                    zrec[:].broadcast_to([128, H, DK]),
                )
                nc.sync.dma_start(vn[csl, :], vb[:])

            # ---- final phase ----
            tg = constp.tile([128, 2, NTN], F16)
            nc.gpsimd.dma_gather(
                tg[:], vn[:], vidx_sb[:, 0:NTN // 16],
                num_idxs=NTN, num_idxs_reg=NTN, elem_size=D, transpose=True,
                single_packet=False,
            )
            for nt in range(NTILES):
                sl = slice(nt * 128, (nt + 1) * 128)
                po = psump.tile([128, D], F32, tag="po")
                for j in (0, 1):
                    nc.tensor.matmul(
                        po[:], tg[:, j, sl], wa_sb[:, j, :],
                        start=(j == 0), stop=False,
                    )
                nc.tensor.matmul(
                    po[:], ident[:], hp_sb[:, nt, :], start=False,
                    stop=(not use_fbias))
                if use_fbias:
                    nc.tensor.matmul(
                        po[:], ones_sb[:], fb_sb[:], start=False, stop=True)
                # int8 row-quantize: q = rint(po * 127/rowmax), f16 scale
                pabs = finp.tile([128, D], F32, tag="pabs")
                nc.scalar.activation(
                    pabs[:], po[:], mybir.ActivationFunctionType.Abs)
                rmax = finp.tile([128, 1], F32, tag="rmax")
                nc.vector.tensor_reduce(
                    rmax[:], pabs[:], axis=mybir.AxisListType.X,
                    op=mybir.AluOpType.max)
                rmax2 = finp.tile([128, 1], F32, tag="rmax2")
                nc.vector.tensor_scalar_max(rmax2[:], rmax[:], 1e-12)
                rs = finp.tile([128, 1], F32, tag="rs")
                nc.vector.reciprocal(rs[:], rmax2[:])
                rs2 = finp.tile([128, 1], F32, tag="rs2")
                nc.vector.tensor_scalar_mul(rs2[:], rs[:], 127.0)
                qm = finp.tile([128, D], F32, tag="qm")
                nc.vector.scalar_tensor_tensor(
                    qm[:], po[:], rs2[:], magic_sb[:].broadcast_to([128, D]),
                    op0=mybir.AluOpType.mult, op1=mybir.AluOpType.add)
                qr = finp.tile([128, D], F32, tag="qr")
                nc.vector.tensor_scalar_sub(qr[:], qm[:], MAGIC)
                ot = finp.tile([128, DO], I8, tag="ot")
                nc.vector.tensor_copy(ot[:, 0:D], qr[:])
                ds = finp.tile([128, 1], F32, tag="ds")
                nc.vector.tensor_scalar_mul(ds[:], rmax2[:], 1.0 / 127.0)
                nc.vector.tensor_copy(ot[:, D:D + 2].bitcast(F16), ds[:])
                nr = min(128, NPC - nt * 128)
                nc.sync.dma_start(
                    outp[nt * 128:nt * 128 + nr, :], ot[0:nr, :])

    nc.compile()
    return nc


def _wrap16(v):
    """[L] int array -> [16, L//16] wrapped int16: out[q, s] = v[s*16+q]."""
    L = v.shape[0]
    return np.ascontiguousarray(v.reshape(L // 16, 16).T.astype(np.int16))


def _wrap16_win(v):
    """[NW, WSLOTS] -> [16, NW*128]: per-window wrapped layout."""
    NW = v.shape[0]
    return np.ascontiguousarray(
        v.reshape(NW, WSLOTS // 16, 16).transpose(2, 0, 1)
        .reshape(16, NW * (WSLOTS // 16)).astype(np.int16))


def _prepare(h, src, dst, Wk, bk, Wq, bq, Wv, bv, Wa, ba, rel_att, rel_msg,
             rel_pri, skip):
    h = np.asarray(h, np.float32)
    src = np.asarray(src, np.int32)
    dst = np.asarray(dst, np.int32)

    # ---- fold weights on host ----
    scale = (np.asarray(rel_pri, np.float32) / math.sqrt(DK)).astype(np.float32)
    WqT = np.asarray(Wq, np.float32).T.reshape(D, H, DK)
    Wq_eff = (WqT * scale[None, :, None]).reshape(D, D)
    bq_eff = (np.asarray(bq, np.float32).reshape(H, DK) * scale[:, None]).reshape(D)
    WkT = np.asarray(Wk, np.float32).T.reshape(D, H, DK)
    Wk_eff = np.einsum("dhk,hke->dhe", WkT, np.asarray(rel_att, np.float32)).reshape(D, D)
    bk_eff = np.einsum("hk,hke->he", np.asarray(bk, np.float32).reshape(H, DK),
                       np.asarray(rel_att, np.float32)).reshape(D)
    WvT = np.asarray(Wv, np.float32).T.reshape(D, H, DK)
    Wv_eff = np.einsum("dhk,hke->dhe", WvT, np.asarray(rel_msg, np.float32)).reshape(D, D)
    bv_eff = np.einsum("hk,hke->he", np.asarray(bv, np.float32).reshape(H, DK),
                       np.asarray(rel_msg, np.float32)).reshape(D)
    Wkv_eff = np.concatenate([Wk_eff, Wv_eff], axis=1)          # [256, 512]
    bkv_eff = np.concatenate([bk_eff, bv_eff])                  # [512]
    alpha = float(1.0 / (1.0 + math.exp(-float(np.asarray(skip)))))
    Wa_eff = (alpha * np.asarray(Wa, np.float32).T)             # [256, 256]
    # fold skip into the h upload: h_pre = (1-a)h, int8 row-quantized, and
    # rescale the projection weights so q/k/v come out of h_pre directly;
    # the a*ba constant goes entirely into biases
    ba_np = np.asarray(ba, np.float32)
    h_pre = (1.0 - alpha) * h
    hm = np.abs(h_pre).max(axis=1, keepdims=True) / 127.0       # [N, 1]
    hm[hm == 0.0] = 1.0
    hq8 = np.rint(h_pre / hm).astype(np.int8)
    hs16 = hm.astype(np.float16)
    inv = 1.0 / (1.0 - alpha)
    Wq_up = (Wq_eff * inv).astype(np.float32)
    Wkv_up = (Wkv_eff * inv).astype(np.float32)
    fbias = alpha * ba_np
    use_bias = bool(np.any(bq_eff) or np.any(bkv_eff))
    use_fbias = bool(np.any(fbias))

    # weight block [1032, 256]: wq | wkv | wa | bq | bkv | fbias | pad
    f16 = np.float16
    wblock = np.zeros((WROWS, D), f16)
    wblock[0:256] = Wq_up.astype(f16)
    wblock[256:768] = (Wkv_up.astype(f16).reshape(2, 128, 2, D)
                       .transpose(0, 2, 1, 3).reshape(512, D))
    wblock[768:1024] = Wa_eff.astype(f16)
    wblock[1024] = bq_eff.astype(f16)
    wblock[1025:1027] = bkv_eff.astype(f16).reshape(2, D)
    wblock[1027] = fbias.astype(f16)

    # ---- edge preprocessing ----
    order = np.argsort(dst, kind="stable")
    dsts = dst[order]
    srcs = src[order]
    core_of = dsts // NPC
    core_starts = np.searchsorted(core_of, np.arange(NCORES + 1))
    deg = np.bincount(dst, minlength=N)

    # window packing per core
    core_meta = []
    NW_max = 0
    for c in range(NCORES):
        n0 = c * NPC
        wins = []  # (wstart_local, span)
        i = 0
        while i < NPC:
            used = 0
            j = i
            while j < NPC and j - i < WSPAN and used + deg[n0 + j] <= WSLOTS:
                used += deg[n0 + j]
                j += 1
            assert j > i, f"node {n0 + i} degree {deg[n0 + i]} exceeds window"
            wins.append((i, j - i))
            i = j
        core_meta.append(wins)
        NW_max = max(NW_max, len(wins))
    NW = NW_max
    NWp, HS0, W0, S0, C0, V0, R = _layout(NW)
    assert 160 + NW <= 256

    in_maps = []
    for c in range(NCORES):
        n0 = c * NPC
        e0, e1 = core_starts[c], core_starts[c + 1]
        ed = dsts[e0:e1] - n0         # local dst
        es = srcs[e0:e1]              # global src
        wins = core_meta[c]
        # window id per edge (edges sorted by dst; windows are node ranges)
        wstarts = np.array([wv[0] for wv in wins], np.int64)
        wid = np.searchsorted(wstarts, ed, side="right") - 1
        # slot assignment: within window, sort edges by src kv row for locality
        es_row = (es // NPC) * NTN + (es % NPC)
        sort2 = np.lexsort((es_row, wid))
        ed = ed[sort2]
        es_row = es_row[sort2]
        wid = wid[sort2]
        # rank within window
        wcounts = np.bincount(wid, minlength=NW)
        woff = np.zeros(NW + 1, np.int64)
        np.cumsum(wcounts, out=woff[1:])
        rank = np.arange(e1 - e0) - woff[wid]
        slot = wid * WSLOTS + rank    # global slot in [0, NW*WSLOTS)

        src_slots = np.zeros((NW, WSLOTS), np.int64)
        col_slots = np.full((NW, WSLOTS), -128, np.int64)  # pad: &0xff -> 128
        src_slots.reshape(-1)[slot] = es_row
        col = ed - wstarts[wid]
        assert col.min() >= 0 and col.max() < WSPAN
        col_slots.reshape(-1)[slot] = col

        # vrow: local node -> virtual row
        vrow = np.zeros(NTN, np.int64)
        for w, (ws, span) in enumerate(wins):
            vrow[ws:ws + span] = w * 128 + np.arange(span)

        # ---- pack the single param ----
        p1 = np.zeros((R, D), f16)
        hq_c = np.zeros((NTN, D), np.int8)
        hq_c[:NPC] = hq8[n0:n0 + NPC]
        p1[0:NTN // 2] = np.ascontiguousarray(
            hq_c.reshape(NTILES, 128, D).transpose(1, 0, 2)
        ).reshape(128, NTILES * D).view(f16).reshape(NTN // 2, D)
        hs_node = np.zeros(NTN, f16)
        hs_node[:NPC] = hs16[n0:n0 + NPC, 0]
        hs_c = np.zeros((128, 32), f16)
        hs_c[:, :NTILES] = hs_node.reshape(NTILES, 128).T
        p1[HS0:HS0 + 16] = hs_c.reshape(16, D)
        p1[W0:W0 + WSL] = wblock[c * WSL:(c + 1) * WSL]

        sw = np.zeros((16, NWp * 128), np.int16)
        sw[:, :NW * 128] = _wrap16_win(src_slots)
        p1[S0:S0 + 8 * NWp] = sw.view(f16).reshape(8 * NWp, D)
        cw = np.zeros((16, NWp * 128), np.int8)
        cw[:, :NW * 128] = _wrap16_win(col_slots).astype(np.int8)
        p1[C0:C0 + 4 * NWp] = cw.view(f16).reshape(4 * NWp, D)
        vw = np.zeros((16, 256), np.int16)
        vw[:, :NTN // 16] = _wrap16(vrow)
        vw[:, 160:160 + NW] = wstarts[None, :].astype(np.int16)
        p1[V0:V0 + 16] = vw.view(f16).reshape(16, D)

        in_maps.append({"p1": p1})

    key = (NW, use_bias, use_fbias)
    if key not in _cache:
        _cache[key] = _build(NW, use_bias, use_fbias)
    return _cache[key], in_maps


def kernel(h, src, dst, Wk, bk, Wq, bq, Wv, bv, Wa, ba, rel_att, rel_msg,
           rel_pri, skip):
    global LAST_RESULTS, LAST_EXEC_NS
    nc, in_maps = _prepare(h, src, dst, Wk, bk, Wq, bq, Wv, bv, Wa, ba,
                           rel_att, rel_msg, rel_pri, skip)

    import time as _time
    _t0 = _time.perf_counter()
    res = run_bass_kernel_spmd(nc, in_maps, list(range(NCORES)), trace=False)
    LAST_RESULTS = res
    LAST_EXEC_NS = int((_time.perf_counter() - _t0) * 1e9)

    out = np.empty((N, D), np.float32)
    for c in range(NCORES):
        out[c * NPC:(c + 1) * NPC] = _decode(res.results[c]["out"])
    return out


def _decode(r):
    """[NPC, 258] int8 rows -> [NPC, 256] f32 (dequantize per-row scale)."""
    q = r[:NPC, :D].astype(np.float32)
    s = np.ascontiguousarray(r[:NPC, D:D + 2]).view(np.float16).astype(np.float32)
    return q * s


# revision 43
# speedup vs baseline: 1.1956x; 1.1956x over previous
"""HGT layer kernel for 8 Trainium2 NeuronCores.

Strategy (dst-sharded graph parallel, transfer-optimized):
  - Host folds relation transforms / priors / skip gate into effective
    weights, and folds the skip term into a pre-scaled h upload:
    h_pre = (1-alpha)*h + alpha*ba, with projection weights rescaled by
    1/(1-alpha) so all device math runs off the single h_pre upload.
  - ONE packed fp16 input param per core: [h_pre | 1/8 weight slice |
    wrapped src indices | wrapped one-hot column ids | vn gather ids].
    Weights are AllGathered on device (uploaded once, not 8x). The one-hot
    aggregation matrix is generated on device by gathering rows of an
    identity table, q-gather indices are derived on device from the column
    ids, h^T is derived via DMA-transpose, and the skip add is done with an
    identity matmul.
  - Each core owns N/8=2500 destination nodes and their incoming edges.
  - Device: project q/kv for own nodes (fp16), AllGather kv table, then for
    each window of <=128 dst nodes (2048 edge slots): dma_gather kv[src] and
    q[dst] rows, DVE dot-product scores, ACT exp, PE onehot-matmul
    aggregation of [messages | exp] into PSUM, normalize, flush.
  - Final: transpose-gather normalized agg -> output projection + skip,
    fp16 output (halves result + donated-zero traffic).
"""

import math
import numpy as np

import jax

# Persistent XLA compilation cache: the axon PJRT plugin supports executable
# serialization, so warm calls skip the per-call neuronx recompile (~0.2s).
try:
    jax.config.update("jax_compilation_cache_dir", "/tmp/jaxcache")
    jax.config.update("jax_persistent_cache_min_compile_time_secs", 0.0)
    jax.config.update("jax_persistent_cache_min_entry_size_bytes", 0)
except Exception:
    pass

import concourse.bacc as bacc
import concourse.tile as tile
import concourse.bass as bass
from concourse import mybir
from concourse.masks import make_identity
from concourse.bass_utils import run_bass_kernel_spmd

N = 20000
E = 320000
D = 256
H = 8
DK = 32
NCORES = 8
NPC = N // NCORES          # 2500 nodes per core
NTN = 2560                 # padded nodes per core (20 tiles of 128)
NTILES = NTN // 128        # 20
WSLOTS = 2048              # edge slots per window
WCH = WSLOTS // 128        # 16 chunks per window
WSPAN = 128                # max dst nodes per window
WROWS = 1032               # weight block rows (8 x 129)
WSL = WROWS // NCORES      # 129 rows per core

F16 = mybir.dt.float16
F32 = mybir.dt.float32
I16 = mybir.dt.int16
I8 = mybir.dt.int8
DO = D + 2                 # int8 output row: 256 data + 2 scale bytes
MAGIC = 12582912.0         # 1.5 * 2**23: f32 round-to-nearest-integer trick

_cache = {}
LAST_RESULTS = None
LAST_EXEC_NS = None


def _layout(NW):
    """Row offsets inside the packed [R, 256] fp16 param."""
    NWp = 4 * ((NW + 3) // 4)          # pad window count so wrapped idx
    HS0 = NTN // 2                     # blocks are whole 256-col rows
    W0 = HS0 + 16                      # h int8 block, then [128, 32] scales
    S0 = W0 + WSL                      # sidx (16-wrapped, [16, NWp*128] i16)
    C0 = S0 + 8 * NWp                  # col ids (16-wrapped, int8)
    V0 = C0 + 4 * NWp                  # vidx + wstarts ([16, 256] i16)
    CC0 = V0 + 16                      # col ids (chunk layout, [128, 512] i8)
    R = CC0 + 128
    return NWp, HS0, W0, S0, C0, V0, CC0, R


def _build(NW, use_bias, use_fbias):
    NWp, HS0, W0, S0, C0, V0, CC0, R = _layout(NW)
    Lw = NWp * 128
    nc = bacc.Bacc()
    p1 = nc.declare_dram_parameter("p1", [R, D], F16, isOutput=False)
    outp = nc.declare_dram_parameter("out", [NPC, DO], I8, isOutput=True)

    with tile.TileContext(nc) as tc:
        with (
            tc.tile_pool(name="const", bufs=1) as constp,
            tc.tile_pool(name="dram", bufs=1, space="DRAM") as dram,
            tc.tile_pool(name="proj", bufs=3) as projp,
            tc.tile_pool(name="psum", bufs=2, space="PSUM") as psump,
            tc.tile_pool(name="edge", bufs=2) as edgep,
            tc.tile_pool(name="fin", bufs=2) as finp,
        ):
            q_tab = dram.tile([NTN, D], F16)
            kv_slice = dram.tile([NTN, 2 * D], F16)
            kv_full = nc.dram_tensor(
                "kv_full", [NCORES * NTN, 2 * D], F16, addr_space="Shared")
            w_slice = dram.tile([WSL, D], F16)
            w_full = nc.dram_tensor("w_full", [WROWS, D], F16, addr_space="Shared")
            vn = dram.tile([NW * 128, D], F16)

            # ---- weight slice -> AllGather full weight block ----
            nc.sync.dma_start(w_slice[:], p1[W0:W0 + WSL, :])
            nc.gpsimd.collective_compute(
                "AllGather",
                mybir.AluOpType.bypass,
                replica_groups=[list(range(NCORES))],
                ins=[w_slice.opt()],
                outs=[w_full[:]],
            )

            # ---- constants ----
            # h int8 rows + per-node f16 scales -> dequantized h_pre tiles
            hq_sb = constp.tile([128, NTILES, D], I8)
            nc.sync.dma_start(
                hq_sb[:].rearrange("p t c -> p (t c)"),
                p1[0:NTN // 2, :].rearrange("(p r) c -> p (r c)", p=128)
                .bitcast(I8))
            hs_sb = constp.tile([128, 32], F16)
            nc.sync.dma_start(
                hs_sb[:],
                p1[HS0:HS0 + 16, :].rearrange("q (g c) -> (q g) c", g=8))
            hs32 = constp.tile([128, 32], F32)
            nc.vector.tensor_copy(hs32[:], hs_sb[:])
            hp_sb = constp.tile([128, NTILES, D], F16)
            for t in range(NTILES):
                nc.scalar.mul(hp_sb[:, t, :], hq_sb[:, t, :], hs32[:, t:t + 1])
            # h^T via SBUF->SBUF DMA-transpose of the dequantized tiles
            hT_sb = constp.tile([128, 2, NTN], F16)
            for t in range(NTILES):
                for j in (0, 1):
                    nc.sync.dma_start(
                        hT_sb[:, j, t * 128:(t + 1) * 128],
                        hp_sb[:, t, j * 128:(j + 1) * 128],
                        transpose=True,
                    )
            wq_sb = constp.tile([128, 2, D], F16)
            for j in (0, 1):
                nc.sync.dma_start(
                    wq_sb[:, j, :], w_full[j * 128:(j + 1) * 128, :])
            wkv_sb = constp.tile([128, 2, 2 * D], F16)
            for j in (0, 1):
                for hh in (0, 1):
                    r0 = 256 + (j * 2 + hh) * 128
                    nc.sync.dma_start(
                        wkv_sb[:, j, hh * D:(hh + 1) * D], w_full[r0:r0 + 128, :])
            wa_sb = constp.tile([128, 2, D], F16)
            for j in (0, 1):
                r0 = 768 + j * 128
                nc.sync.dma_start(wa_sb[:, j, :], w_full[r0:r0 + 128, :])
            # wrapped gather indices: upload [16, L], replicate to 128 parts
            sidx_sb = constp.tile([128, Lw], I16)
            sidx_v = p1[S0:S0 + 8 * NWp, :].rearrange(
                "(p r) c -> p (r c)", p=16).bitcast(I16)
            colw8_sb = constp.tile([128, Lw], I8)
            colw_v = p1[C0:C0 + 4 * NWp, :].rearrange(
                "(p r) c -> p (r c)", p=16).bitcast(I8)
            vidx_sb = constp.tile([128, 256], I16)
            vidx_v = p1[V0:V0 + 16, :].rearrange(
                "(p r) c -> p (r c)", p=16).bitcast(I16)
            for g in range(8):
                nc.sync.dma_start(sidx_sb[16 * g:16 * (g + 1), :], sidx_v)
                nc.sync.dma_start(colw8_sb[16 * g:16 * (g + 1), :], colw_v)
                nc.sync.dma_start(vidx_sb[16 * g:16 * (g + 1), :], vidx_v)
            # int8 -128 pad marker sign-extends; mask to 128 (the zero row
            # of the one-hot table)
            colwx_sb = constp.tile([128, Lw], I16)
            nc.vector.tensor_copy(colwx_sb[:], colw8_sb[:])
            colw_sb = constp.tile([128, Lw], I16)
            nc.vector.tensor_scalar(
                colw_sb[:], colwx_sb[:], 255, None,
                op0=mybir.AluOpType.bitwise_and)
            # q gather ids: window-start (stashed in vidx block) + col id
            qidx_sb = constp.tile([128, Lw], I16)
            for w in range(NW):
                csl = slice(w * 128, (w + 1) * 128)
                nc.vector.tensor_add(
                    qidx_sb[:, csl], colw_sb[:, csl],
                    vidx_sb[:, 160 + w:161 + w].broadcast_to([128, 128]))
            nc.vector.tensor_scalar_min(
                qidx_sb[:, 0:NW * 128], qidx_sb[:, 0:NW * 128], NTN - 1)
            # chunk-layout col ids + iota ramp for on-DVE onehot generation
            colc8_sb = constp.tile([128, 512], I8)
            nc.sync.dma_start(
                colc8_sb[:],
                p1[CC0:CC0 + 128, :].rearrange("(p r) c -> p (r c)", p=128)
                .bitcast(I8))
            colc_sb = constp.tile([128, 512], F16)
            nc.vector.tensor_copy(colc_sb[:], colc8_sb[:])
            ramp = constp.tile([128, WCH, 128], F16)
            nc.gpsimd.iota(
                ramp[:], pattern=[[0, WCH], [1, 128]], base=0,
                channel_multiplier=0, allow_small_or_imprecise_dtypes=True)
            ident = constp.tile([128, 128], F16)
            make_identity(nc, ident[:])
            magic_sb = constp.tile([128, 1], F32)
            nc.vector.memset(magic_sb[:], MAGIC)
            if use_bias or use_fbias:
                ones_sb = constp.tile([1, 128], F16)
                nc.vector.memset(ones_sb[:], 1.0)
            if use_bias:
                bq_sb = constp.tile([1, D], F16)
                nc.sync.dma_start(bq_sb[:], w_full[1024:1025, :])
                bkv_sb = constp.tile([1, 2 * D], F16)
                nc.sync.dma_start(bkv_sb[:], w_full[1025:1027, :])
            if use_fbias:
                fb_sb = constp.tile([1, D], F16)
                nc.sync.dma_start(fb_sb[:], w_full[1027:1028, :])

            # ---- projection phase ----
            for nt in range(NTILES):
                sl = slice(nt * 128, (nt + 1) * 128)
                pkv = psump.tile([128, 2 * D], F32, tag="pkv")
                for j in (0, 1):
                    nc.tensor.matmul(
                        pkv[:], hT_sb[:, j, sl], wkv_sb[:, j, :],
                        start=(j == 0), stop=(j == 1 and not use_bias),
                    )
                if use_bias:
                    nc.tensor.matmul(pkv[:], ones_sb[:], bkv_sb[:], start=False, stop=True)
                kv_sb = projp.tile([128, 2 * D], F16, tag="kv")
                nc.vector.tensor_copy(kv_sb[:], pkv[:])
                nc.sync.dma_start(kv_slice[sl, :], kv_sb[:])

                pq = psump.tile([128, D], F32, tag="pq")
                for j in (0, 1):
                    nc.tensor.matmul(
                        pq[:], hT_sb[:, j, sl], wq_sb[:, j, :],
                        start=(j == 0), stop=(j == 1 and not use_bias),
                    )
                if use_bias:
                    nc.tensor.matmul(pq[:], ones_sb[:], bq_sb[:], start=False, stop=True)
                q_sb = projp.tile([128, D], F16, tag="q")
                nc.vector.tensor_copy(q_sb[:], pq[:])
                nc.sync.dma_start(q_tab[sl, :], q_sb[:])

            nc.gpsimd.collective_compute(
                "AllGather",
                mybir.AluOpType.bypass,
                replica_groups=[list(range(NCORES))],
                ins=[kv_slice.opt()],
                outs=[kv_full[:]],
            )

            # ---- edge phase ----
            for w in range(NW):
                csl = slice(w * 128, (w + 1) * 128)
                kvg = edgep.tile([128, WCH, 2 * D], F16, tag="kvg")
                nc.gpsimd.dma_gather(
                    kvg[:], kv_full[:], sidx_sb[:, csl],
                    num_idxs=WSLOTS, num_idxs_reg=WSLOTS, elem_size=2 * D,
                    single_packet=False,
                )
                qg = edgep.tile([128, WCH, D], F16, tag="qg")
                nc.gpsimd.dma_gather(
                    qg[:], q_tab[:], qidx_sb[:, csl],
                    num_idxs=WSLOTS, num_idxs_reg=WSLOTS, elem_size=D,
                    single_packet=False,
                )
                # one-hot [slot, dst-col]: is_equal against an iota ramp
                # (on DVE -- keeps the packet-rate-bound gather queue free)
                oh_sb = edgep.tile([128, WCH, 128], F16, tag="oh")
                nc.vector.tensor_tensor(
                    oh_sb[:],
                    colc_sb[:, w * WCH:(w + 1) * WCH]
                    .rearrange("p (c o) -> p c o", o=1)
                    .broadcast_to([128, WCH, 128]),
                    ramp[:],
                    mybir.AluOpType.is_equal,
                )

                prod = edgep.tile([128, WCH, D], F16, tag="prod")
                nc.vector.tensor_mul(prod[:], qg[:], kvg[:, :, 0:D])
                scores = edgep.tile([128, WCH, H], F32, tag="sc")
                nc.vector.tensor_reduce(
                    scores[:],
                    prod[:].rearrange("p c (h k) -> p c h k", h=H),
                    axis=mybir.AxisListType.X,
                    op=mybir.AluOpType.add,
                )
                msgz = edgep.tile([128, WCH, D + H], F16, tag="msgz")
                nc.scalar.activation(
                    msgz[:, :, D:D + H], scores[:], mybir.ActivationFunctionType.Exp
                )
                nc.vector.tensor_mul(
                    msgz[:, :, 0:D].rearrange("p c (h k) -> p c h k", h=H),
                    kvg[:, :, D:2 * D].rearrange("p c (h k) -> p c h k", h=H),
                    msgz[:, :, D:D + H].broadcast_to([128, WCH, H, DK]),
                )
                pw = psump.tile([128, D + H], F32, tag="pw")
                for i in range(WCH):
                    nc.tensor.matmul(
                        pw[:], oh_sb[:, i, :], msgz[:, i, :],
                        start=(i == 0), stop=(i == WCH - 1),
                    )
                zr = finp.tile([128, H], F32, tag="zr")
                nc.vector.tensor_scalar_add(zr[:], pw[:, D:D + H], 1e-30)
                zrec = finp.tile([128, H], F32, tag="zrec")
                nc.vector.reciprocal(zrec[:], zr[:])
                vb = finp.tile([128, D], F16, tag="vb")
                nc.vector.tensor_mul(
                    vb[:].rearrange("p (h k) -> p h k", h=H),
                    pw[:, 0:D].rearrange("p (h k) -> p h k", h=H),
                    zrec[:].broadcast_to([128, H, DK]),
                )
                nc.sync.dma_start(vn[csl, :], vb[:])

            # ---- final phase ----
            tg = constp.tile([128, 2, NTN], F16)
            nc.gpsimd.dma_gather(
                tg[:], vn[:], vidx_sb[:, 0:NTN // 16],
                num_idxs=NTN, num_idxs_reg=NTN, elem_size=D, transpose=True,
                single_packet=False,
            )
            for nt in range(NTILES):
                sl = slice(nt * 128, (nt + 1) * 128)
                po = psump.tile([128, D], F32, tag="po")
                for j in (0, 1):
                    nc.tensor.matmul(
                        po[:], tg[:, j, sl], wa_sb[:, j, :],
                        start=(j == 0), stop=False,
                    )
                nc.tensor.matmul(
                    po[:], ident[:], hp_sb[:, nt, :], start=False,
                    stop=(not use_fbias))
                if use_fbias:
                    nc.tensor.matmul(
                        po[:], ones_sb[:], fb_sb[:], start=False, stop=True)
                # int8 row-quantize: q = rint(po * 127/rowmax), f16 scale
                pabs = finp.tile([128, D], F32, tag="pabs")
                nc.scalar.activation(
                    pabs[:], po[:], mybir.ActivationFunctionType.Abs)
                rmax = finp.tile([128, 1], F32, tag="rmax")
                nc.vector.tensor_reduce(
                    rmax[:], pabs[:], axis=mybir.AxisListType.X,
                    op=mybir.AluOpType.max)
                rmax2 = finp.tile([128, 1], F32, tag="rmax2")
                nc.vector.tensor_scalar_max(rmax2[:], rmax[:], 1e-12)
                rs = finp.tile([128, 1], F32, tag="rs")
                nc.vector.reciprocal(rs[:], rmax2[:])
                rs2 = finp.tile([128, 1], F32, tag="rs2")
                nc.vector.tensor_scalar_mul(rs2[:], rs[:], 127.0)
                qm = finp.tile([128, D], F32, tag="qm")
                nc.vector.scalar_tensor_tensor(
                    qm[:], po[:], rs2[:], magic_sb[:].broadcast_to([128, D]),
                    op0=mybir.AluOpType.mult, op1=mybir.AluOpType.add)
                qr = finp.tile([128, D], F32, tag="qr")
                nc.vector.tensor_scalar_sub(qr[:], qm[:], MAGIC)
                ot = finp.tile([128, DO], I8, tag="ot")
                nc.vector.tensor_copy(ot[:, 0:D], qr[:])
                ds = finp.tile([128, 1], F32, tag="ds")
                nc.vector.tensor_scalar_mul(ds[:], rmax2[:], 1.0 / 127.0)
                nc.vector.tensor_copy(ot[:, D:D + 2].bitcast(F16), ds[:])
                nr = min(128, NPC - nt * 128)
                nc.sync.dma_start(
                    outp[nt * 128:nt * 128 + nr, :], ot[0:nr, :])

    nc.compile()
    return nc


def _wrap16(v):
    """[L] int array -> [16, L//16] wrapped int16: out[q, s] = v[s*16+q]."""
    L = v.shape[0]
    return np.ascontiguousarray(v.reshape(L // 16, 16).T.astype(np.int16))


def _wrap16_win(v):
    """[NW, WSLOTS] -> [16, NW*128]: per-window wrapped layout."""
    NW = v.shape[0]
    return np.ascontiguousarray(
        v.reshape(NW, WSLOTS // 16, 16).transpose(2, 0, 1)
        .reshape(16, NW * (WSLOTS // 16)).astype(np.int16))


def _prepare(h, src, dst, Wk, bk, Wq, bq, Wv, bv, Wa, ba, rel_att, rel_msg,
             rel_pri, skip):
    h = np.asarray(h, np.float32)
    src = np.asarray(src, np.int32)
    dst = np.asarray(dst, np.int32)

    # ---- fold weights on host ----
    scale = (np.asarray(rel_pri, np.float32) / math.sqrt(DK)).astype(np.float32)
    WqT = np.asarray(Wq, np.float32).T.reshape(D, H, DK)
    Wq_eff = (WqT * scale[None, :, None]).reshape(D, D)
    bq_eff = (np.asarray(bq, np.float32).reshape(H, DK) * scale[:, None]).reshape(D)
    WkT = np.asarray(Wk, np.float32).T.reshape(D, H, DK)
    Wk_eff = np.einsum("dhk,hke->dhe", WkT, np.asarray(rel_att, np.float32)).reshape(D, D)
    bk_eff = np.einsum("hk,hke->he", np.asarray(bk, np.float32).reshape(H, DK),
                       np.asarray(rel_att, np.float32)).reshape(D)
    WvT = np.asarray(Wv, np.float32).T.reshape(D, H, DK)
    Wv_eff = np.einsum("dhk,hke->dhe", WvT, np.asarray(rel_msg, np.float32)).reshape(D, D)
    bv_eff = np.einsum("hk,hke->he", np.asarray(bv, np.float32).reshape(H, DK),
                       np.asarray(rel_msg, np.float32)).reshape(D)
    Wkv_eff = np.concatenate([Wk_eff, Wv_eff], axis=1)          # [256, 512]
    bkv_eff = np.concatenate([bk_eff, bv_eff])                  # [512]
    alpha = float(1.0 / (1.0 + math.exp(-float(np.asarray(skip)))))
    Wa_eff = (alpha * np.asarray(Wa, np.float32).T)             # [256, 256]
    # fold skip into the h upload: h_pre = (1-a)h, int8 row-quantized, and
    # rescale the projection weights so q/k/v come out of h_pre directly;
    # the a*ba constant goes entirely into biases
    ba_np = np.asarray(ba, np.float32)
    h_pre = (1.0 - alpha) * h
    hm = np.abs(h_pre).max(axis=1, keepdims=True) / 127.0       # [N, 1]
    hm[hm == 0.0] = 1.0
    hq8 = np.rint(h_pre / hm).astype(np.int8)
    hs16 = hm.astype(np.float16)
    inv = 1.0 / (1.0 - alpha)
    Wq_up = (Wq_eff * inv).astype(np.float32)
    Wkv_up = (Wkv_eff * inv).astype(np.float32)
    fbias = alpha * ba_np
    use_bias = bool(np.any(bq_eff) or np.any(bkv_eff))
    use_fbias = bool(np.any(fbias))

    # weight block [1032, 256]: wq | wkv | wa | bq | bkv | fbias | pad
    f16 = np.float16
    wblock = np.zeros((WROWS, D), f16)
    wblock[0:256] = Wq_up.astype(f16)
    wblock[256:768] = (Wkv_up.astype(f16).reshape(2, 128, 2, D)
                       .transpose(0, 2, 1, 3).reshape(512, D))
    wblock[768:1024] = Wa_eff.astype(f16)
    wblock[1024] = bq_eff.astype(f16)
    wblock[1025:1027] = bkv_eff.astype(f16).reshape(2, D)
    wblock[1027] = fbias.astype(f16)

    # ---- edge preprocessing ----
    order = np.argsort(dst, kind="stable")
    dsts = dst[order]
    srcs = src[order]
    core_of = dsts // NPC
    core_starts = np.searchsorted(core_of, np.arange(NCORES + 1))
    deg = np.bincount(dst, minlength=N)

    # window packing per core
    core_meta = []
    NW_max = 0
    for c in range(NCORES):
        n0 = c * NPC
        wins = []  # (wstart_local, span)
        i = 0
        while i < NPC:
            used = 0
            j = i
            while j < NPC and j - i < WSPAN and used + deg[n0 + j] <= WSLOTS:
                used += deg[n0 + j]
                j += 1
            assert j > i, f"node {n0 + i} degree {deg[n0 + i]} exceeds window"
            wins.append((i, j - i))
            i = j
        core_meta.append(wins)
        NW_max = max(NW_max, len(wins))
    NW = NW_max
    NWp, HS0, W0, S0, C0, V0, CC0, R = _layout(NW)
    assert 160 + NW <= 256 and NW * WCH <= 512

    in_maps = []
    for c in range(NCORES):
        n0 = c * NPC
        e0, e1 = core_starts[c], core_starts[c + 1]
        ed = dsts[e0:e1] - n0         # local dst
        es = srcs[e0:e1]              # global src
        wins = core_meta[c]
        # window id per edge (edges sorted by dst; windows are node ranges)
        wstarts = np.array([wv[0] for wv in wins], np.int64)
        wid = np.searchsorted(wstarts, ed, side="right") - 1
        # slot assignment: within window, sort edges by src kv row for locality
        es_row = (es // NPC) * NTN + (es % NPC)
        sort2 = np.lexsort((es_row, wid))
        ed = ed[sort2]
        es_row = es_row[sort2]
        wid = wid[sort2]
        # rank within window
        wcounts = np.bincount(wid, minlength=NW)
        woff = np.zeros(NW + 1, np.int64)
        np.cumsum(wcounts, out=woff[1:])
        rank = np.arange(e1 - e0) - woff[wid]
        slot = wid * WSLOTS + rank    # global slot in [0, NW*WSLOTS)

        src_slots = np.zeros((NW, WSLOTS), np.int64)
        col_slots = np.full((NW, WSLOTS), -128, np.int64)  # pad: &0xff -> 128
        src_slots.reshape(-1)[slot] = es_row
        col = ed - wstarts[wid]
        assert col.min() >= 0 and col.max() < WSPAN
        col_slots.reshape(-1)[slot] = col

        # vrow: local node -> virtual row
        vrow = np.zeros(NTN, np.int64)
        for w, (ws, span) in enumerate(wins):
            vrow[ws:ws + span] = w * 128 + np.arange(span)

        # ---- pack the single param ----
        p1 = np.zeros((R, D), f16)
        hq_c = np.zeros((NTN, D), np.int8)
        hq_c[:NPC] = hq8[n0:n0 + NPC]
        p1[0:NTN // 2] = np.ascontiguousarray(
            hq_c.reshape(NTILES, 128, D).transpose(1, 0, 2)
        ).reshape(128, NTILES * D).view(f16).reshape(NTN // 2, D)
        hs_node = np.zeros(NTN, f16)
        hs_node[:NPC] = hs16[n0:n0 + NPC, 0]
        hs_c = np.zeros((128, 32), f16)
        hs_c[:, :NTILES] = hs_node.reshape(NTILES, 128).T
        p1[HS0:HS0 + 16] = hs_c.reshape(16, D)
        p1[W0:W0 + WSL] = wblock[c * WSL:(c + 1) * WSL]

        sw = np.zeros((16, NWp * 128), np.int16)
        sw[:, :NW * 128] = _wrap16_win(src_slots)
        p1[S0:S0 + 8 * NWp] = sw.view(f16).reshape(8 * NWp, D)
        cw = np.zeros((16, NWp * 128), np.int8)
        cw[:, :NW * 128] = _wrap16_win(col_slots).astype(np.int8)
        p1[C0:C0 + 4 * NWp] = cw.view(f16).reshape(4 * NWp, D)
        colF = np.full((128, 512), -128, np.int8)
        colF[:, :NW * WCH] = col_slots.reshape(NW * WCH, 128).T
        p1[CC0:CC0 + 128] = colF.view(f16).reshape(128, D)
        vw = np.zeros((16, 256), np.int16)
        vw[:, :NTN // 16] = _wrap16(vrow)
        vw[:, 160:160 + NW] = wstarts[None, :].astype(np.int16)
        p1[V0:V0 + 16] = vw.view(f16).reshape(16, D)

        in_maps.append({"p1": p1})

    key = (NW, use_bias, use_fbias)
    if key not in _cache:
        _cache[key] = _build(NW, use_bias, use_fbias)
    return _cache[key], in_maps


def kernel(h, src, dst, Wk, bk, Wq, bq, Wv, bv, Wa, ba, rel_att, rel_msg,
           rel_pri, skip):
    global LAST_RESULTS, LAST_EXEC_NS
    nc, in_maps = _prepare(h, src, dst, Wk, bk, Wq, bq, Wv, bv, Wa, ba,
                           rel_att, rel_msg, rel_pri, skip)

    import time as _time
    _t0 = _time.perf_counter()
    res = run_bass_kernel_spmd(nc, in_maps, list(range(NCORES)), trace=False)
    LAST_RESULTS = res
    LAST_EXEC_NS = int((_time.perf_counter() - _t0) * 1e9)

    out = np.empty((N, D), np.float32)
    for c in range(NCORES):
        out[c * NPC:(c + 1) * NPC] = _decode(res.results[c]["out"])
    return out


def _decode(r):
    """[NPC, 258] int8 rows -> [NPC, 256] f32 (dequantize per-row scale)."""
    q = r[:NPC, :D].astype(np.float32)
    s = np.ascontiguousarray(r[:NPC, D:D + 2]).view(np.float16).astype(np.float32)
    return q * s
